# revision 2
# baseline (speedup 1.0000x reference)
"""Dense dot-product attention (score = Q@V^T, softmax, context = A@V) on 8
TRN2 NeuronCores, batch-parallel: each core owns B/8 = 2 batches.

Per batch on one core (Lq = Lkv = 1024, D = 512, fp32 I/O):
  - Q, V loaded in natural [l, d] layout (fast contiguous DMA).
  - QT/VT ([d, l], needed because the PE contracts over the partition dim)
    are produced by PE transpose-mode matmuls; the PSUM->SBUF copy rounds to
    float32r, which runs matmuls at 1 cycle/row (4x fp32) with ~13-bit
    mantissa accuracy - measured score RMS error 3e-3, far better than bf16.
  - S = QT.T @ VT accumulated in PSUM per 128-row q-tile.
  - Softmax per q-tile: DVE reduce_max (negated) -> ACT exp(S - max) with
    fused row-sum, writing E in fp16 -> DVE reciprocal.
  - E^T for the second matmul via one DMA xbar block-transpose per q-tile
    (dest[p, kt, j] = E[j, kt*128 + p]).
  - context = (E^T.T @ V_fp16) * (1/Z); attn = E * (1/Z).
The emission order software-pipelines mm1 two q-tiles ahead so the PE never
waits for the softmax chain.
"""
import sys

sys.path.insert(0, "/opt/trn_rl_repo")

import collections
from contextlib import ExitStack

import numpy as np

import concourse.bass as bass
import concourse.tile as tile
from concourse import mybir
from concourse.bass_utils import run_bass_kernel_spmd

F32 = mybir.dt.float32
F32R = mybir.dt.float32r
F16 = mybir.dt.float16

N_CORES = 8
B, LQ, LKV, D = 16, 1024, 1024, 512
BPC = B // N_CORES  # batches per core
NQT = LQ // 128  # q-tiles per batch
NKT = LKV // 128  # k-tiles per batch
NDT = D // 128  # d-tiles


# --- post-Tile pass: hardware wait-slot limits -------------------------------
# Engine instructions carry a single hardware semaphore-wait slot; Tile's
# sem-assigner sometimes emits more. Hoist excess waits onto single-wait NOPs
# spliced immediately before the instruction on the same engine (the NX
# sequencer dispatches in order, so the NOPs block until the sems clear).
_WAIT_LIMITS = collections.defaultdict(lambda: 1)


def _fix_wait_limits(nc):
    n_fixed = 0
    for fn in nc.m.functions:
        for blk in fn.blocks:
            out = []
            for inst in blk.instructions:
                limit = _WAIT_LIMITS[type(inst).__name__]
                si = inst.sync_info
                if si is not None and si.on_wait and len(si.on_wait) > limit:
                    hoist = list(si.on_wait)[: len(si.on_wait) - limit]
                    keep = list(si.on_wait)[len(si.on_wait) - limit :]
                    for i, w in enumerate(hoist):
                        out.append(
                            mybir.InstNoOp(
                                name=f"{inst.name}-waitnop{i}",
                                engine=inst.engine,
                                sync_info=mybir.SyncInfo(on_wait=[w], on_update=[]),
                                bass_nofuse=True,
                            )
                        )
                    inst.sync_info = mybir.SyncInfo(
                        on_wait=keep, on_update=list(si.on_update or [])
                    )
                    n_fixed += 1
                out.append(inst)
            blk.instructions = out
    return n_fixed


def build():
    nc = bass.Bass("TRN2", target_bir_lowering=False, debug=False)
    q = nc.dram_tensor("query", [BPC, LQ, D], F32, kind="ExternalInput").ap()
    v = nc.dram_tensor("value", [BPC, LKV, D], F32, kind="ExternalInput").ap()
    iden = nc.dram_tensor("iden", [128, 128], F32, kind="ExternalInput").ap()
    ctx_out = nc.dram_tensor("context", [BPC, LQ, D], F32, kind="ExternalOutput").ap()
    attn_out = nc.dram_tensor("attn", [BPC, LQ, LKV], F32, kind="ExternalOutput").ap()

    with ExitStack() as ctx:
        tc = ctx.enter_context(tile.TileContext(nc))
        singles = ctx.enter_context(tc.tile_pool(name="singles", bufs=1))
        iop = ctx.enter_context(tc.tile_pool(name="io", bufs=2))
        tp = ctx.enter_context(tc.tile_pool(name="tp", bufs=2))
        ep = ctx.enter_context(tc.tile_pool(name="ep", bufs=2))
        sp = ctx.enter_context(tc.tile_pool(name="sp", bufs=8))
        # PSUM: S tiles are 2 banks each (3 bufs = 6 banks); C-psum and the
        # transpose staging tiles are 1 bank each and share the last 2 banks
        # via a common tag (temporally mostly disjoint).
        pss = ctx.enter_context(tc.tile_pool(name="pss", bufs=3, space="PSUM"))
        psc = ctx.enter_context(tc.tile_pool(name="psc", bufs=2, space="PSUM"))

        ident = singles.tile([128, 128], F32)
        nc.sync.dma_start(ident[:], iden)

        # All HBM loads issued up front so they never queue behind stores.
        qn = {}
        vn = {}
        for b in range(BPC):
            qn[b] = iop.tile([128, NQT, D], F32, tag="qn", name=f"qn{b}")
            nc.sync.dma_start(qn[b][:], q[b].rearrange("(t p) d -> p t d", p=128))
            vn[b] = iop.tile([128, NKT, D], F32, tag="vn", name=f"vn{b}")
            nc.sync.dma_start(vn[b][:], v[b].rearrange("(t p) d -> p t d", p=128))

        for b in range(BPC):
            # fp16 V for the second matmul's moving operand
            vh = tp.tile([128, NKT, D], F16, tag="vh")
            for kt in range(NKT):
                nc.vector.tensor_copy(vh[:, kt, :], vn[b][:, kt, :])

            # QT/VT via PE transpose; copies round fp32 -> fp32r.
            # QT copies on DVE, VT copies on ACT to balance engine load.
            qt = tp.tile([128, NDT, LQ], F32R, tag="qt")  # [d_lo, dt, q]
            vt = tp.tile([128, NDT, LKV], F32R, tag="vt")  # [d_lo, dt, k]
            for src, dst, eng in ((qn[b], qt, "v"), (vn[b], vt, "s")):
                nt = NQT if src is qn[b] else NKT
                for dt in range(NDT):
                    for g in range(nt // 4):
                        pst = psc.tile([128, 512], F32, tag="pb")
                        for j in range(4):
                            blk = g * 4 + j
                            nc.tensor.transpose(
                                pst[:, j * 128 : (j + 1) * 128],
                                src[:, blk, dt * 128 : (dt + 1) * 128],
                                ident[:],
                            )
                        dslice = dst[:, dt, g * 512 : (g + 1) * 512]
                        if eng == "v":
                            nc.vector.tensor_copy(dslice, pst[:])
                        else:
                            nc.scalar.copy(dslice, pst[:])

            # Per q-tile pipeline, mm1 emitted two tiles ahead of mm2.
            spsum = {}

            def mm1(qb):
                s = pss.tile([128, LKV], F32, tag="s")
                for kc in range(LKV // 512):
                    for dt in range(NDT):
                        nc.tensor.matmul(
                            s[:, kc * 512 : (kc + 1) * 512],
                            qt[:, dt, qb * 128 : (qb + 1) * 128],
                            vt[:, dt, kc * 512 : (kc + 1) * 512],
                            start=(dt == 0),
                            stop=(dt == NDT - 1),
                        )
                spsum[qb] = s

            def softmax_mm2(qb):
                s = spsum.pop(qb)
                nmx = sp.tile([128, 1], F32, tag="nmx")
                nc.vector.reduce_max(
                    nmx[:], s[:], axis=mybir.AxisListType.X, negate=True
                )
                e16 = ep.tile([128, LKV], F16, tag="e16")
                zsum = sp.tile([128, 1], F32, tag="zsum")
                nc.scalar.activation(
                    e16[:],
                    s[:],
                    mybir.ActivationFunctionType.Exp,
                    bias=nmx[:],
                    scale=1.0,
                    accum_out=zsum[:],
                )
                rz = sp.tile([128, 1], F32, tag="rz")
                nc.vector.reciprocal(rz[:], zsum[:])
                # E^T blocks: one xbar transpose; et[p, kt, j] = e16[j, kt*128+p]
                et = ep.tile([128, NKT, 128], F16, tag="et")
                nc.sync.dma_start(et[:], e16[:], transpose=True)
                # attn row block = E * (1/Z)
                a32 = ep.tile([128, LKV], F32, tag="a32")
                nc.scalar.activation(
                    a32[:],
                    e16[:],
                    mybir.ActivationFunctionType.Copy,
                    scale=rz[:],
                )
                nc.sync.dma_start(
                    attn_out[b, qb * 128 : (qb + 1) * 128, :], a32[:]
                )
                # context = (E^T.T @ V) * (1/Z)
                cps = psc.tile([128, D], F32, tag="pb")
                for kt in range(NKT):
                    nc.tensor.matmul(
                        cps[:],
                        et[:, kt, :],
                        vh[:, kt, :],
                        start=(kt == 0),
                        stop=(kt == NKT - 1),
                    )
                c32 = ep.tile([128, D], F32, tag="c32")
                nc.vector.tensor_scalar_mul(c32[:], cps[:], rz[:])
                nc.sync.dma_start(
                    ctx_out[b, qb * 128 : (qb + 1) * 128, :], c32[:]
                )

            mm1(0)
            mm1(1)
            for qb in range(NQT):
                if qb + 2 < NQT:
                    mm1(qb + 2)
                softmax_mm2(qb)

    _fix_wait_limits(nc)
    return nc


_NC = None


def _get_nc():
    global _NC
    if _NC is None:
        _NC = build()
    return _NC


_IDEN = np.eye(128, dtype=np.float32)


def kernel(query: np.ndarray, value: np.ndarray):
    query = np.ascontiguousarray(query, dtype=np.float32)
    value = np.ascontiguousarray(value, dtype=np.float32)
    nc = _get_nc()
    in_maps = [
        {
            "query": query[c * BPC : (c + 1) * BPC],
            "value": value[c * BPC : (c + 1) * BPC],
            "iden": _IDEN,
        }
        for c in range(N_CORES)
    ]
    res = run_bass_kernel_spmd(nc, in_maps, core_ids=list(range(N_CORES)))
    context = np.concatenate([r["context"] for r in res.results], axis=0)
    attn = np.concatenate([r["attn"] for r in res.results], axis=0)
    return context, attn
